# revision 3
# baseline (speedup 1.0000x reference)
"""AWQ quantized linear (4096 -> 11008) on 8 trn2 NeuronCores.

Column-parallel sharding: each core owns OUT/8 = 1376 output features.
Per core:
  - unpack int32-packed nibbles (low/high interleaved) on DVE
  - per-group (128-wide) affine dequant via tensor_scalar (per-partition scalars)
  - PE-transpose weight tiles to [IN, OUT_SH] fp16 resident in SBUF, folding the
    per-input-channel inv_scale into the PSUM->SBUF copy (ACT)
  - x path: SWDGE cast-DMA f32->fp16 into per-block internal DRAM tiles, then
    hardware DMA-transpose (xbar) loads xT k-chunk tiles straight into SBUF
  - fp16 matmuls accumulate in f32 PSUM over 32 K-chunks; N-slices 512/512/352
  - bias added during PSUM->SBUF copy; f32 stores
"""

import sys

for _p in ("/opt/trn_rl_repo", "/opt/pypackages"):
    if _p not in sys.path:
        sys.path.append(_p)

import numpy as np

import concourse.bass as bass
import concourse.mybir as mybir
import concourse.tile as tile
from concourse import bacc
from concourse.bass_utils import run_bass_kernel_spmd
from concourse.masks import make_identity

IN = 4096
OUT = 11008
N_CORES = 8
OUT_SH = OUT // N_CORES  # 1376
T = 8192
NK = IN // 128  # 32 k-chunks
P = 128
TB = 512  # token block for DMA-transpose staging

dt = mybir.dt
Alu = mybir.AluOpType
Act = mybir.ActivationFunctionType


def build(n_t_tiles=T // P, out_sh=OUT_SH):
    n_o_tiles = (out_sh + P - 1) // P
    nsl = []
    n0 = 0
    while n0 < out_sh:
        nsz = min(512, out_sh - n0)
        nsl.append((n0, nsz))
        n0 += nsz

    n_tok = n_t_tiles * P
    tb = min(TB, n_tok)
    n_blocks = (n_tok + tb - 1) // tb
    tiles_per_block = tb // P

    nc = bacc.Bacc("TRN2", target_bir_lowering=False, debug=False,
                   num_devices=N_CORES)
    x = nc.dram_tensor("x", [n_tok, IN], dt.float32,
                       kind="ExternalInput").ap()
    pk = nc.dram_tensor("pk", [out_sh, IN // 2], dt.int32,
                        kind="ExternalInput").ap()
    sc = nc.dram_tensor("sc", [out_sh, NK], dt.float32,
                        kind="ExternalInput").ap()
    of = nc.dram_tensor("of", [out_sh, NK], dt.float32,
                        kind="ExternalInput").ap()
    inv = nc.dram_tensor("inv", [IN], dt.float32, kind="ExternalInput").ap()
    bias = nc.dram_tensor("bias", [out_sh], dt.float32,
                          kind="ExternalInput").ap()
    out = nc.dram_tensor("out", [n_tok, out_sh], dt.float32,
                         kind="ExternalOutput").ap()

    with tile.TileContext(nc) as tc:
        with (
            tc.tile_pool(name="const", bufs=1) as constp,
            tc.tile_pool(name="wtp", bufs=1) as wtp,
            tc.tile_pool(name="prep", bufs=1) as prep,
            tc.tile_pool(name="prepsm", bufs=4) as prepsm,
            tc.tile_pool(name="dramp", bufs=1, space="DRAM") as dramp,
            tc.tile_pool(name="xtp", bufs=2) as xtp,
            tc.tile_pool(name="outp", bufs=2) as outp,
            tc.tile_pool(name="pmm", bufs=2 * len(nsl), space="PSUM") as pmm,
            tc.tile_pool(name="ptp", bufs=2, space="PSUM") as ptp,
        ):
            ident = constp.tile([P, P], dt.float32)
            make_identity(nc, ident[:])

            inv_sb = constp.tile([P, NK], dt.float32)
            nc.sync.dma_start(inv_sb[:], inv.rearrange("(c p) -> p c", p=P))

            bias_bc = constp.tile([P, out_sh], dt.float32)
            nc.sync.dma_start(bias_bc[:], bias[None, :].to_broadcast([P, out_sh]))

            # fp16 W^T, laid out [128 (i within chunk), NK chunks, out_sh]
            wt = wtp.tile([P, NK, out_sh], dt.float16)

            # ---- x cast pass: f32 -> fp16 into per-block DRAM tiles (SWDGE) ----
            x16 = [dramp.tile([tb, IN], dt.float16, name=f"x16_{b}")
                   for b in range(n_blocks)]
            for b in range(n_blocks):
                nc.gpsimd.dma_start(x16[b][:], x[b * tb:(b + 1) * tb, :])

            # ---- weight prep ----
            for ot in range(n_o_tiles):
                rows = min(P, out_sh - ot * P)
                o0 = ot * P
                pkt = prep.tile([P, IN // 2], dt.int32, tag="pkt")
                nc.sync.dma_start(pkt[:rows], pk[o0:o0 + rows, :])
                sct = prep.tile([P, NK], dt.float32, tag="sct")
                nc.sync.dma_start(sct[:rows], sc[o0:o0 + rows, :])
                oft = prep.tile([P, NK], dt.float32, tag="oft")
                nc.sync.dma_start(oft[:rows], of[o0:o0 + rows, :])
                for c in range(NK):
                    wq = prepsm.tile([P, 64, 2], dt.int32, tag="wq")
                    src = pkt[:rows, c * 64:(c + 1) * 64]
                    nc.vector.tensor_scalar(wq[:rows, :, 0], src, 15, None,
                                            op0=Alu.bitwise_and)
                    nc.vector.tensor_scalar(wq[:rows, :, 1], src, 4, None,
                                            op0=Alu.logical_shift_right)
                    wd = prepsm.tile([P, P], dt.float32, tag="wd")
                    nc.vector.tensor_scalar(
                        wd[:rows], wq[:rows].rearrange("p a b -> p (a b)"),
                        sct[:rows, c:c + 1], oft[:rows, c:c + 1],
                        op0=Alu.mult, op1=Alu.add)
                    ps = ptp.tile([P, P], dt.float32, tag="tp")
                    nc.tensor.transpose(ps[:, :rows], wd[:rows, :],
                                        ident[:rows, :rows])
                    nc.scalar.activation(wt[:, c, o0:o0 + rows], ps[:, :rows],
                                         Act.Copy, scale=inv_sb[:, c:c + 1])

            # ---- main loop over token blocks ----
            for b in range(n_blocks):
                xtb = xtp.tile([P, NK, tb], dt.float16, tag="xtb")
                for c in range(NK):
                    nc.sync.dma_start(xtb[:, c, :],
                                      x16[b][:, c * P:(c + 1) * P],
                                      transpose=True)
                for it in range(tiles_per_block):
                    tt = b * tiles_per_block + it
                    t0 = tt * P
                    po = [pmm.tile([P, 512], dt.float32, tag="po",
                                   name=f"po{tt}_{j}")
                          for j in range(len(nsl))]
                    for c in range(NK):
                        for j, (n0, nsz) in enumerate(nsl):
                            nc.tensor.matmul(
                                po[j][:, :nsz],
                                lhsT=xtb[:, c, it * P:(it + 1) * P],
                                rhs=wt[:, c, n0:n0 + nsz],
                                start=(c == 0), stop=(c == NK - 1))
                    osb = outp.tile([P, out_sh], dt.float32, tag="osb")
                    for j, (n0, nsz) in enumerate(nsl):
                        nc.vector.tensor_add(osb[:, n0:n0 + nsz],
                                             po[j][:, :nsz],
                                             bias_bc[:, n0:n0 + nsz])
                    nc.sync.dma_start(out[t0:t0 + P, :], osb[:])

    nc.compile()
    return nc


def make_in_maps(x, packed, scales, offsets, inv_scale, bias, out_sh=OUT_SH):
    xf = np.ascontiguousarray(np.asarray(x, dtype=np.float32).reshape(-1, IN))
    pkm = np.asarray(packed, dtype=np.int32).reshape(OUT, IN // 2)
    scm = np.asarray(scales, dtype=np.float32).reshape(OUT, NK)
    ofm = np.asarray(offsets, dtype=np.float32).reshape(OUT, NK)
    invv = np.asarray(inv_scale, dtype=np.float32)
    bv = np.asarray(bias, dtype=np.float32)
    in_maps = []
    for k in range(N_CORES):
        sl = slice(k * out_sh, (k + 1) * out_sh)
        in_maps.append({
            "x": xf,
            "pk": np.ascontiguousarray(pkm[sl]),
            "sc": np.ascontiguousarray(scm[sl]),
            "of": np.ascontiguousarray(ofm[sl]),
            "inv": invv,
            "bias": np.ascontiguousarray(bv[sl]),
        })
    return in_maps


_CACHE = {}


def kernel(x, packed, scales, offsets, inv_scale, bias):
    if "nc" not in _CACHE:
        _CACHE["nc"] = build()
    nc = _CACHE["nc"]
    in_maps = make_in_maps(x, packed, scales, offsets, inv_scale, bias)
    res = run_bass_kernel_spmd(nc, in_maps, list(range(N_CORES)))
    cols = [res.results[k]["out"] for k in range(N_CORES)]
    full = np.concatenate(cols, axis=1)
    return np.ascontiguousarray(full.reshape(4, 2048, OUT).astype(np.float32))
